# revision 49
# baseline (speedup 1.0000x reference)
"""Causal multi-head attention (B=4, T=2048, D=1024, H=16, HD=64) on 8
Trainium2 NeuronCores.

Sharding: data-parallel over batch (4) x tensor-parallel over heads (2
groups of 8). Each core runs the same Bass program on its own input
slices; the host sums the two tensor-parallel partial projections per
batch and adds b_proj.

v3-v10 (382us v2 -> ~283us on this machine; rel err 3.5e-3):
  - all DMA'd inputs (xT/wq/wk/wv) in bf16: halves DMA bytes and,
    critically, halves LDWEIGHTS so weight loads hide under the 512-col
    matmuls (QKV matmuls 303ns -> 216ns = full 2.4GHz clock).  Also
    avoids the f32r ap<256 4x penalty on diagonal partial S matmuls.
    Total dtype error ~4.5e-3 vs the 2e-2 budget.
  - startup: K chains first (their DVE bias-adds gate kT for S and the
    smps bufs for V, so they drain under the Q matmuls), k-major
    emission for chunk-0 chains (all 4 accumulators track the weight
    DMA stream), wk/wq on the ACT HWDGE queue, xT/wv/wp on SP, wpool
    sized for both weight sets in flight.
  - softmax finalize decoupled: DVE drains av PSUM (values -> scr,
    ones-row denominator -> a base-partition-0 tile; the custom-DVE
    recip reads garbage from base partition 64 on HW - CoreSim does
    not model it) so the banks free early; recip -> GpSimd broadcast
    -> scale runs off the PE critical path into osb.
  - proj chains spread: proj(_, cp) fills attn(cp+1) rather than all
    crowding attn3; the first 4 tail chains emit their k<3 partials
    before any osb[3]-dependent matmul.
  - group-start fills forced to >=2 PE-work units so the PE has
    matmul work while exp(b0) drains the first strip buffer.

Per-core dataflow (feature-major, no on-chip transposes):
  xT [D,T] (host pre-transposed)
  Q^T,K^T = w-stationary matmuls -> [512, T] feature-major (bf16)
  V       = xT-stationary matmuls -> [T, 512+ones] token-major bf16
  S^T     = K^T-block-stationary matmuls, 2 heads packed in the 128-deep
            PE array (contraction = hd = 64, row groups 0/64)
  P       = exp(S^T) on ScalarE -> bf16 (1/8 scale folded into wq)
  O^T,den = V|1-stationary matmuls accumulating over tk blocks (ones
            column yields the softmax denominator in PSUM row 64)
  out     = O^T * partition_broadcast(1/den)
  yT      = wp-stationary matmuls -> [D, T] partial (host reduces)
"""

import numpy as np
import ml_dtypes

import concourse.bass as bass
import concourse.bacc as bacc
import concourse.mybir as mybir
import concourse.tile as tile
from concourse.bass_utils import run_bass_kernel_spmd

F32 = mybir.dt.float32
F32R = mybir.dt.float32r
BF16 = mybir.dt.bfloat16
AF = mybir.ActivationFunctionType

B, T, D = 4, 2048, 1024
H, HD = 16, 64
NH = 8          # heads per core
DL = NH * HD    # 512 local qkv feature dim
CH = T // 512   # 4 chunks of 512 tokens
KT = T // 128   # 16 tk blocks
VW = 65         # V columns per head incl. ones column


def build(nc: bass.Bass):
    xT = nc.declare_dram_parameter("xT", [D, T], BF16, isOutput=False)
    wq = nc.declare_dram_parameter("wq", [D, DL], BF16, isOutput=False)
    wk = nc.declare_dram_parameter("wk", [D, DL], BF16, isOutput=False)
    wv = nc.declare_dram_parameter("wv", [D, DL], BF16, isOutput=False)
    bq = nc.declare_dram_parameter("bq", [DL], F32, isOutput=False)
    bk = nc.declare_dram_parameter("bk", [DL], F32, isOutput=False)
    bv = nc.declare_dram_parameter("bv", [DL], F32, isOutput=False)
    wp = nc.declare_dram_parameter("wp", [DL, D], BF16, isOutput=False)
    tri = nc.declare_dram_parameter("tri", [128, 256], BF16, isOutput=False)
    yT = nc.declare_dram_parameter("yT", [D, T], F32, isOutput=True)

    with tile.TileContext(nc) as tc:
        with (
            tc.tile_pool(name="persist", bufs=1) as persist,
            tc.tile_pool(name="wpool", bufs=16) as wpool,
            tc.tile_pool(name="qtp", bufs=6) as qtp,
            tc.tile_pool(name="xtp", bufs=16) as xtp,
            tc.tile_pool(name="etp", bufs=8) as etp,
            tc.tile_pool(name="recp", bufs=2) as recp,
            tc.tile_pool(name="scrp", bufs=4) as scrp,
            tc.tile_pool(name="ytp", bufs=6) as ytp,
            tc.tile_pool(name="bigps", bufs=2, space="PSUM") as bigps,
            tc.tile_pool(name="smps", bufs=4, space="PSUM") as smps,
        ):
            # -------- persistent tiles --------
            kT = [persist.tile([128, T], BF16, tag=f"kt{i}", name=f"kt{i}")
                  for i in range(4)]
            v_sb = [persist.tile([128, NH * VW], BF16, tag=f"v{i}",
                                 name=f"v{i}") for i in range(KT)]
            osb = [persist.tile([128, T], BF16, tag=f"o{i}", name=f"o{i}")
                   for i in range(4)]
            wv_sb = [persist.tile([128, DL], BF16, tag=f"wv{k}",
                                  name=f"wv{k}") for k in range(8)]
            wp_sb = [persist.tile([128, D], BF16, tag=f"wp{k}",
                                  name=f"wp{k}") for k in range(4)]
            ones64 = persist.tile([1, 64], F32, tag="ones64", name="ones64")
            bq_sb = persist.tile([128, 4], F32, tag="bq", name="bq_sb")
            bk_sb = persist.tile([128, 4], F32, tag="bk", name="bk_sb")
            bv_bc = persist.tile([128, DL], F32, tag="bv", name="bv_bc")
            tri_sb = persist.tile([128, 256], BF16, tag="tri", name="tri_sb")

            def load_small_consts():
                nc.sync.dma_start(
                    out=bq_sb, in_=bq[:].rearrange("(a p) -> p a", p=128)
                )
                nc.sync.dma_start(
                    out=bk_sb, in_=bk[:].rearrange("(a p) -> p a", p=128)
                )
                nc.sync.dma_start(
                    out=bv_bc,
                    in_=bass.AP(tensor=bv[:].tensor, offset=0,
                                ap=[[0, 128], [1, DL]]),
                )
                nc.sync.dma_start(out=tri_sb, in_=tri[:, :])
                nc.vector.memset(ones64, 1.0)
                # ones columns of V (written once; V drains leave them)
                for tt in range(KT):
                    v3 = v_sb[tt].rearrange("p (h c) -> p h c", c=VW)
                    nc.vector.memset(v3[:, :, HD:HD + 1], 1.0)

            def load_big_consts():
                # wv streams on the SP queue behind xT; wp on the ACT queue
                # behind wk/wq (proj weights are needed much later)
                for k in range(8):
                    nc.sync.dma_start(
                        out=wv_sb[k], in_=wv[128 * k: 128 * k + 128, :]
                    )
                # wp on SP too: the ACT queue must drain before the first
                # exp can issue, and proj doesn't need wp until much later
                for k in range(4):
                    nc.sync.dma_start(
                        out=wp_sb[k], in_=wp[128 * k: 128 * k + 128, :]
                    )

            qts = [[None] * 4 for _ in range(CH)]

            # ---------------- QKV chain emitters ----------------
            def xt_unit(c):
                cs = slice(512 * c, 512 * c + 512)
                xts = []
                for k in range(8):
                    t_ = xtp.tile([128, 512], BF16, tag="xt", name="xt")
                    nc.sync.dma_start(
                        out=t_, in_=xT[128 * k: 128 * k + 128, cs]
                    )
                    xts.append(t_)
                return xts

            def w_unit(w_in, eng=None):
                eng = eng or nc.sync
                wts = []
                for k in range(8):
                    wt = wpool.tile([128, DL], BF16, tag="w", name="wt")
                    eng.dma_start(
                        out=wt, in_=w_in[128 * k: 128 * k + 128, :]
                    )
                    wts.append(wt)
                return wts

            def qk_chain(c, n, xts, wts, b_sb, is_q):
                acc = smps.tile([128, 512], F32, tag="sm", name="acc")
                for k in range(8):
                    nc.tensor.matmul(
                        acc,
                        wts[k][:, 128 * n: 128 * n + 128],
                        xts[k],
                        start=(k == 0), stop=(k == 7),
                    )
                qk_store(c, n, acc, b_sb, is_q)

            def qk_store(c, n, acc, b_sb, is_q):
                if is_q:
                    qt_n = qtp.tile([128, 512], BF16, tag="qt", name="qt")
                    nc.vector.tensor_scalar_add(
                        out=qt_n, in0=acc, scalar1=b_sb[:, n: n + 1]
                    )
                    qts[c][n] = qt_n
                else:
                    cs = slice(512 * c, 512 * c + 512)
                    nc.vector.tensor_scalar_add(
                        out=kT[n][:, cs], in0=acc, scalar1=b_sb[:, n: n + 1]
                    )

            def qk_chains_kmajor(c, xts, wts, b_sb, is_q):
                # all 4 chains advance together: at the k-th DMA landing,
                # 4 matmuls run, so the chunk-0 chains track the weight
                # stream instead of serializing chain-by-chain
                accs = [smps.tile([128, 512], F32, tag="sm", name="acc")
                        for _ in range(4)]
                for k in range(8):
                    for n in range(4):
                        nc.tensor.matmul(
                            accs[n],
                            wts[k][:, 128 * n: 128 * n + 128],
                            xts[k],
                            start=(k == 0), stop=(k == 7),
                        )
                for n in range(4):
                    qk_store(c, n, accs[n], b_sb, is_q)

            def v_chain(c, t4, xts):
                tt = 4 * c + t4
                acc = smps.tile([128, 512], F32, tag="sm", name="acc")
                for k in range(8):
                    nc.tensor.matmul(
                        acc,
                        xts[k][:, 128 * t4: 128 * t4 + 128],
                        wv_sb[k],
                        start=(k == 0), stop=(k == 7),
                    )
                v3 = v_sb[tt].rearrange("p (h c) -> p h c", c=VW)
                a3 = acc.rearrange("p (h c) -> p h c", c=HD)
                b3 = bv_bc.rearrange("p (h c) -> p h c", c=HD)
                nc.vector.tensor_add(v3[:, :, 0:HD], a3, b3)

            def qkv_units(c):
                """Emission units for chunk c's QKV; each is a callable."""
                state = {}

                def u_xw():
                    # interleave x and wq blocks on the SP queue so the
                    # k-th Q matmul's operands land together and the chain
                    # pipelines with the DMA stream instead of stalling on
                    # wq queued behind all 8 x blocks
                    cs = slice(512 * c, 512 * c + 512)
                    xts, wts = [], []
                    for k in range(8):
                        t_ = xtp.tile([128, 512], BF16, tag="xt", name="xt")
                        nc.sync.dma_start(
                            out=t_, in_=xT[128 * k: 128 * k + 128, cs])
                        xts.append(t_)
                        wt = wpool.tile([128, DL], BF16, tag="w", name="wt")
                        nc.sync.dma_start(
                            out=wt, in_=wq[128 * k: 128 * k + 128, :])
                        wts.append(wt)
                    state["xts"] = xts
                    state["wq"] = wts

                def u_wk():
                    state["wk"] = w_unit(wk)

                # units tagged (fn, has_pe_work): forced fills count only
                # PE-compute units (a DMA-trigger unit doesn't cover a PE
                # stall).  The DMA units lead the list so a single forced
                # fill at the chunk transition gets every stream going
                # ~15us earlier than pacing would.
                units = [(u_xw, False), (u_wk, False)]
                for n in range(4):
                    units.append((
                        lambda n=n: qk_chain(c, n, state["xts"],
                                             state["wq"], bq_sb, True),
                        True))
                for n in range(4):
                    units.append((
                        lambda n=n: qk_chain(c, n, state["xts"],
                                             state["wk"], bk_sb, False),
                        True))
                for t4 in range(4):
                    units.append((
                        lambda t4=t4: v_chain(c, t4, state["xts"]),
                        True))
                return units

            taccs = {}

            def tacc_partial(n):
                # first 3 contraction blocks of a tail proj chain; emitted
                # as late attn3 fillers (osb[0..2] chunk-3 columns are final
                # after groups 0..2), finished in the tail once osb[3] lands
                acc = smps.tile([128, 512], F32, tag="sm", name="acc")
                for k in range(3):
                    nc.tensor.matmul(
                        acc,
                        wp_sb[k][:, 128 * n: 128 * n + 128],
                        osb[k][:, 1536:2048],
                        start=(k == 0), stop=False,
                    )
                taccs[n] = acc

            def proj_chain(n, cp):
                acc = smps.tile([128, 512], F32, tag="sm", name="acc")
                for k in range(4):
                    nc.tensor.matmul(
                        acc,
                        wp_sb[k][:, 128 * n: 128 * n + 128],
                        osb[k][:, 512 * cp: 512 * cp + 512],
                        start=(k == 0), stop=(k == 3),
                    )
                yt = ytp.tile([128, 512], F32, tag="yt", name="yt")
                nc.vector.tensor_copy(yt, acc)
                nc.sync.dma_start(
                    out=yT[128 * n: 128 * n + 128,
                           512 * cp: 512 * cp + 512],
                    in_=yt,
                )

            # ---------------- attention ----------------
            def attn_group(g2, c, fill):
                qt, kt = qts[c][g2], kT[g2]
                ha, hb = 2 * g2, 2 * g2 + 1
                nb = 4 * (c + 1)
                av_a = smps.tile([VW, 512], F32, tag="sm", name="av_a")
                av_b = smps.tile([VW, 512], F32, tag="sm", name="av_b")
                ets = []

                def boff(b):
                    return 128 * (b - 4 * c) if b // 4 == c else 0

                strips = {}

                def s_mms(b):
                    off = boff(b)
                    bs = slice(128 * b, 128 * b + 128)
                    strip = bigps.tile([128, 1024], F32, tag="strip",
                                       name="strip")
                    strips[b] = strip
                    nc.tensor.matmul(
                        strip[:, off:512],
                        kt[0:64, bs],
                        qt[0:64, off:512],
                        start=True, stop=True,
                    )
                    nc.tensor.matmul(
                        strip[:, 512 + off:1024],
                        kt[64:128, bs],
                        qt[64:128, off:512],
                        start=True, stop=True,
                    )

                def exp_mask(b):
                    off = boff(b)
                    strip = strips.pop(b)
                    et = etp.tile([128, 1024], BF16, tag="et", name="et")
                    if off == 0:
                        nc.scalar.activation(
                            et[:, 0:1024], strip[:, 0:1024], AF.Exp
                        )
                    else:
                        w_ = 512 - off
                        src_ap = bass.AP(
                            tensor=strip.tensor,
                            offset=strip.offset + off,
                            ap=[list(strip.ap[0]), [512, 2], [1, w_]],
                        )
                        dst_ap = bass.AP(
                            tensor=et.tensor,
                            offset=et.offset + off,
                            ap=[list(et.ap[0]), [512, 2], [1, w_]],
                        )
                        nc.scalar.activation(dst_ap, src_ap, AF.Exp)
                    if b // 4 == c:
                        m_dst = bass.AP(
                            tensor=et.tensor,
                            offset=et.offset + off,
                            ap=[list(et.ap[0]), [512, 2], [1, 128]],
                        )
                        m_tri = bass.AP(
                            tensor=tri_sb.tensor,
                            offset=tri_sb.offset,
                            ap=[list(tri_sb.ap[0]), [128, 2], [1, 128]],
                        )
                        nc.vector.tensor_tensor(
                            out=m_dst, in0=m_dst, in1=m_tri,
                            op=mybir.AluOpType.mult,
                        )
                    ets.append(et)

                def av(b):
                    et, off = ets[b], boff(b)
                    nc.tensor.matmul(
                        av_a[:, off:512],
                        v_sb[b][:, VW * ha: VW * ha + VW],
                        et[:, off:512],
                        start=(b == 0), stop=(b == nb - 1),
                    )
                    nc.tensor.matmul(
                        av_b[:, off:512],
                        v_sb[b][:, VW * hb: VW * hb + VW],
                        et[:, 512 + off:1024],
                        start=(b == 0), stop=(b == nb - 1),
                    )

                # blocks processed in pairs: each pair's 4 S matmuls are
                # emitted back-to-back (K=64 row-tiled pairs run packed on
                # disjoint row groups); AV trails by 2 pairs so its exp/mask
                # inputs are always long done when the PE reaches it
                np_ = nb // 2
                for p in range(np_):
                    b0, b1 = 2 * p, 2 * p + 1
                    s_mms(b0)
                    s_mms(b1)
                    exp_mask(b0)
                    exp_mask(b1)
                    if p >= 1:
                        av(2 * p - 2)
                        av(2 * p - 1)
                        fill()
                    else:
                        # force >=2 filler units at the group start so the
                        # PE has matmul work while exp(b0) drains the strip
                        # buffer (only 2 bigps bufs -> s_mms(b2) must wait).
                        # At g2=0 count any units (the DMA-trigger units
                        # must go out anyway) but pump enough to reach the
                        # leading proj unit + the chunk's DMA issues.
                        # g2=0 (chunk transition): proj-lead + the 2 DMA
                        # units for attn1/2; attn0/attn3 have no stall-free
                        # PE lead resp. no DMA units, so just 2.  More
                        # would overflow the 4 smps accumulators (attn3) or
                        # block on just-issued DMAs (attn0).
                        fill(force=(3 if g2 == 0 and 0 < c < CH - 1
                                    else 2),
                             force_any=(g2 == 0))
                av(nb - 2)
                av(nb - 1)
                fill()

                # softmax finalize, decoupled: one DVE copy per head drains
                # the av bank (64 value rows + ones-row denominator) to SBUF
                # scratch so the PSUM bank frees in ~0.8us; the recip ->
                # broadcast -> scale chain then runs out of SBUF off the PE
                # critical path, with no fillers injected mid-chain (a yt
                # copy between the drain and the recip would stretch the
                # last group's osb latency).  Single-pass approx recip
                # (~5e-4 rel) is plenty: den >= 1 and the tolerance budget
                # is 2e-2.
                qs = slice(512 * c, 512 * c + 512)
                scr_a = scrp.tile([HD, 512], F32, tag="scr", name="scr_a")
                scr_b = scrp.tile([HD, 512], F32, tag="scr", name="scr_b")
                den_a = recp.tile([1, 512], F32, tag="den", name="den_a")
                den_b = recp.tile([1, 512], F32, tag="den", name="den_b")
                rec_a = recp.tile([1, 512], F32, tag="rec", name="rec_a")
                rec_b = recp.tile([1, 512], F32, tag="rec", name="rec_b")
                bc_a = recp.tile([64, 512], F32, tag="bc", name="bc_a")
                bc_b = recp.tile([64, 512], F32, tag="bc", name="bc_b")
                # den (ones-row) bounced to a base-partition-0 tile: the
                # custom-DVE recip reads garbage from base partition 64 on
                # hardware (CoreSim doesn't model the quirk)
                if c == CH - 1 and g2 == 3:
                    # last group: every ns of this chain is exposed before
                    # the tail's osb[3] matmuls.  Flush the reserved fillers
                    # (PE work for the window), drain the denominators first
                    # on DVE while the Scalar engine (idle - exps are done)
                    # drains the value rows in parallel, and keep the DVE
                    # queue free of the value drains so recip/mul run early.
                    fill(force=10 ** 6)
                    nc.vector.tensor_copy(den_a, av_a[HD:HD + 1, :])
                    nc.vector.reciprocal_approx_fast(out=rec_a, in_=den_a)
                    nc.gpsimd.partition_broadcast(bc_a, rec_a)
                    nc.vector.tensor_copy(den_b, av_b[HD:HD + 1, :])
                    nc.vector.reciprocal_approx_fast(out=rec_b, in_=den_b)
                    nc.gpsimd.partition_broadcast(bc_b, rec_b)
                    nc.vector.tensor_copy(scr_a, av_a[0:HD, :])
                    nc.vector.tensor_mul(osb[g2][0:64, qs], scr_a, bc_a)
                    nc.vector.tensor_copy(scr_b, av_b[0:HD, :])
                    nc.vector.tensor_mul(osb[g2][64:128, qs], scr_b, bc_b)
                else:
                    # drains first: the av PSUM banks free ~1.2us earlier,
                    # unblocking the next group's av accumulators; the
                    # recip/broadcast/mul chain into osb is off the
                    # critical path for non-last groups
                    nc.vector.tensor_copy(den_a, av_a[HD:HD + 1, :])
                    nc.vector.tensor_copy(scr_a, av_a[0:HD, :])
                    nc.vector.tensor_copy(den_b, av_b[HD:HD + 1, :])
                    nc.vector.tensor_copy(scr_b, av_b[0:HD, :])
                    nc.vector.reciprocal_approx_fast(out=rec_a, in_=den_a)
                    nc.gpsimd.partition_broadcast(bc_a, rec_a)
                    nc.vector.reciprocal_approx_fast(out=rec_b, in_=den_b)
                    nc.gpsimd.partition_broadcast(bc_b, rec_b)
                    nc.vector.tensor_mul(
                        osb[g2][0:64, qs], scr_a, bc_a
                    )
                    nc.vector.tensor_mul(
                        osb[g2][64:128, qs], scr_b, bc_b
                    )
                fill()

            # ---------------- chunk-0 QKV (startup-critical) ----------------
            # K first: its DVE bias-adds (which gate both kT for the S
            # matmuls and the smps bufs for the V chains) drain while the
            # Q matmuls run.  wk+xT stream in parallel on the two HWDGE
            # queues; wq behind wk on ACT; wv behind xT on SP.
            xts0 = xt_unit(0)
            load_small_consts()
            wks0 = w_unit(wk, eng=nc.scalar)
            wqs0 = w_unit(wq, eng=nc.scalar)
            load_big_consts()
            qk_chains_kmajor(0, xts0, wks0, bk_sb, False)
            qk_chains_kmajor(0, xts0, wqs0, bq_sb, True)
            for t4 in range(4):
                v_chain(0, t4, xts0)

            # ---------------- main pipeline ----------------
            for c in range(CH):
                with nc.named_scope(f"attn{c}"):
                    # proj(_, cp) only needs attention chunk cp complete, so
                    # it fills attn(cp+1) instead of crowding attn3.  One
                    # proj unit leads the list: it is the only stall-free PE
                    # unit at the chunk transition (osb + persistent wp), so
                    # the forced fill at g2=0/p=0 can cover the strip wait.
                    fillers = []
                    if c >= 1:
                        fillers += [((lambda n=0, cp=c - 1:
                                      proj_chain(n, cp)), True)]
                    if c < CH - 1:
                        fillers += qkv_units(c + 1)
                    if c >= 1:
                        fillers += [
                            ((lambda n=n, cp=c - 1: proj_chain(n, cp)), True)
                            for n in range(1, 8)
                        ]
                    if c == CH - 1:
                        fillers += [
                            ((lambda n=n: tacc_partial(n)), True)
                            for n in range(2)
                        ]
                    # attn3 over-provisions fill points so ~2 units remain
                    # for the last group's finalize flush (PE starves there
                    # otherwise - exp stream is done, tail waits on osb[3])
                    total_pts = 4 * (2 * (c + 1) + 2)
                    if c == CH - 1:
                        total_pts += 4
                    state = {"pts": 0, "emitted": 0}

                    def fill(state=state, fillers=fillers,
                             total_pts=total_pts, force=0, force_any=False):
                        state["pts"] += 1
                        goal = state["pts"] / total_pts
                        n_ = len(fillers)
                        while (state["emitted"] < n_
                               and (state["emitted"] / n_ <= goal
                                    or force > 0)):
                            fn, is_pe = fillers[state["emitted"]]
                            fn()
                            state["emitted"] += 1
                            if is_pe or force_any:
                                force -= 1

                    for g2 in range(4):
                        attn_group(g2, c, fill)
                    while state["emitted"] < len(fillers):
                        fillers[state["emitted"]][0]()
                        state["emitted"] += 1

            with nc.named_scope("tail"):
                # all 8 chains pre-compute their k<3 partials before any
                # k=3 matmul: n=2,3 reuse the freed av smps banks, n=4..7
                # borrow the bigps strip banks (attention is done - each
                # [128,1024] strip hosts two accumulators), so only the 8
                # k=3 matmuls wait on osb[3]
                tacc_partial(2)
                tacc_partial(3)
                tstrips = [bigps.tile([128, 1024], F32, tag="strip",
                                      name="tstrip") for _ in range(2)]
                for n in range(4, 8):
                    acc = tstrips[(n - 4) // 2][:, 512 * (n % 2):
                                                512 * (n % 2) + 512]
                    for k in range(3):
                        nc.tensor.matmul(
                            acc,
                            wp_sb[k][:, 128 * n: 128 * n + 128],
                            osb[k][:, 1536:2048],
                            start=(k == 0), stop=False,
                        )
                    taccs[n] = acc
                for n in range(8):
                    nc.tensor.matmul(
                        taccs[n],
                        wp_sb[3][:, 128 * n: 128 * n + 128],
                        osb[3][:, 1536:2048],
                        start=False, stop=True,
                    )
                    yt = ytp.tile([128, 512], F32, tag="yt", name="yt")
                    nc.vector.tensor_copy(yt, taccs[n])
                    nc.sync.dma_start(
                        out=yT[128 * n: 128 * n + 128, 1536:2048],
                        in_=yt,
                    )
    return nc


_prog = None


def _get_program():
    global _prog
    if _prog is None:
        _prog = build(bacc.Bacc(None))
        _prog.finalize()
    return _prog


def make_in_maps(x, w_qkv, b_qkv, w_proj):
    x = np.ascontiguousarray(np.asarray(x, np.float32))
    w_qkv = np.asarray(w_qkv, np.float32)
    b_qkv = np.asarray(b_qkv, np.float32)
    w_proj = np.asarray(w_proj, np.float32)
    tri1 = np.triu(np.ones((128, 128), np.float32))
    tri2 = np.concatenate([tri1, tri1], 1).astype(ml_dtypes.bfloat16)
    in_maps = []
    for core in range(8):
        b, g = divmod(core, 2)
        gs = slice(DL * g, DL * g + DL)
        gk = slice(D + DL * g, D + DL * g + DL)
        gv = slice(2 * D + DL * g, 2 * D + DL * g + DL)
        in_maps.append({
            "xT": np.ascontiguousarray(x[b].T).astype(ml_dtypes.bfloat16),
            "wq": (np.ascontiguousarray(w_qkv[:, gs])
                   * np.float32(0.125)).astype(ml_dtypes.bfloat16),
            "wk": np.ascontiguousarray(
                w_qkv[:, gk]).astype(ml_dtypes.bfloat16),
            "wv": np.ascontiguousarray(
                w_qkv[:, gv]).astype(ml_dtypes.bfloat16),
            "bq": np.ascontiguousarray(b_qkv[gs]) * np.float32(0.125),
            "bk": np.ascontiguousarray(b_qkv[gk]),
            "bv": np.ascontiguousarray(b_qkv[gv]),
            "wp": np.ascontiguousarray(
                w_proj[DL * g: DL * g + DL, :]).astype(ml_dtypes.bfloat16),
            "tri": tri2,
        })
    return in_maps


def combine_outputs(results, b_proj):
    b_proj = np.asarray(b_proj, np.float32)
    y = np.empty((B, T, D), np.float32)
    for b in range(B):
        yt = results[2 * b]["yT"] + results[2 * b + 1]["yT"]
        y[b] = yt.T + b_proj
    return y


def kernel(x, w_qkv, b_qkv, w_proj, b_proj, **run_kwargs):
    in_maps = make_in_maps(x, w_qkv, b_qkv, w_proj)
    r = run_bass_kernel_spmd(_get_program(), in_maps,
                             core_ids=list(range(8)), **run_kwargs)
    out = combine_outputs(r.results, b_proj)
    kernel.last_result = r
    return out


# revision 52
# speedup vs baseline: 1.0274x; 1.0274x over previous
"""Causal multi-head attention (B=4, T=2048, D=1024, H=16, HD=64) on 8
Trainium2 NeuronCores.

Sharding: data-parallel over batch (4) x tensor-parallel over heads (2
groups of 8). Each core runs the same Bass program on its own input
slices; the host sums the two tensor-parallel partial projections per
batch and adds b_proj.

v3-v10 (382us v2 -> ~283us on this machine; rel err 3.5e-3):
  - all DMA'd inputs (xT/wq/wk/wv) in bf16: halves DMA bytes and,
    critically, halves LDWEIGHTS so weight loads hide under the 512-col
    matmuls (QKV matmuls 303ns -> 216ns = full 2.4GHz clock).  Also
    avoids the f32r ap<256 4x penalty on diagonal partial S matmuls.
    Total dtype error ~4.5e-3 vs the 2e-2 budget.
  - startup: K chains first (their DVE bias-adds gate kT for S and the
    smps bufs for V, so they drain under the Q matmuls), k-major
    emission for chunk-0 chains (all 4 accumulators track the weight
    DMA stream), wk/wq on the ACT HWDGE queue, xT/wv/wp on SP, wpool
    sized for both weight sets in flight.
  - softmax finalize decoupled: DVE drains av PSUM (values -> scr,
    ones-row denominator -> a base-partition-0 tile; the custom-DVE
    recip reads garbage from base partition 64 on HW - CoreSim does
    not model it) so the banks free early; recip -> GpSimd broadcast
    -> scale runs off the PE critical path into osb.
  - proj chains spread: proj(_, cp) fills attn(cp+1) rather than all
    crowding attn3; the first 4 tail chains emit their k<3 partials
    before any osb[3]-dependent matmul.
  - group-start fills forced to >=2 PE-work units so the PE has
    matmul work while exp(b0) drains the first strip buffer.

Per-core dataflow (feature-major, no on-chip transposes):
  xT [D,T] (host pre-transposed)
  Q^T,K^T = w-stationary matmuls -> [512, T] feature-major (bf16)
  V       = xT-stationary matmuls -> [T, 512+ones] token-major bf16
  S^T     = K^T-block-stationary matmuls, 2 heads packed in the 128-deep
            PE array (contraction = hd = 64, row groups 0/64)
  P       = exp(S^T) on ScalarE -> bf16 (1/8 scale folded into wq)
  O^T,den = V|1-stationary matmuls accumulating over tk blocks (ones
            column yields the softmax denominator in PSUM row 64)
  out     = O^T * partition_broadcast(1/den)
  yT      = wp-stationary matmuls -> [D, T] partial (host reduces)
"""

import numpy as np
import ml_dtypes

import concourse.bass as bass
import concourse.bacc as bacc
import concourse.mybir as mybir
import concourse.tile as tile
from concourse.bass_utils import run_bass_kernel_spmd

F32 = mybir.dt.float32
F32R = mybir.dt.float32r
BF16 = mybir.dt.bfloat16
AF = mybir.ActivationFunctionType

B, T, D = 4, 2048, 1024
H, HD = 16, 64
NH = 8          # heads per core
DL = NH * HD    # 512 local qkv feature dim
CH = T // 512   # 4 chunks of 512 tokens
KT = T // 128   # 16 tk blocks
VW = 65         # V columns per head incl. ones column


def build(nc: bass.Bass):
    xT = nc.declare_dram_parameter("xT", [D, T], BF16, isOutput=False)
    wq = nc.declare_dram_parameter("wq", [D, DL], BF16, isOutput=False)
    wk = nc.declare_dram_parameter("wk", [D, DL], BF16, isOutput=False)
    wv = nc.declare_dram_parameter("wv", [D, DL], BF16, isOutput=False)
    bq = nc.declare_dram_parameter("bq", [DL], F32, isOutput=False)
    bk = nc.declare_dram_parameter("bk", [DL], F32, isOutput=False)
    bv = nc.declare_dram_parameter("bv", [DL], F32, isOutput=False)
    wp = nc.declare_dram_parameter("wp", [DL, D], BF16, isOutput=False)
    tri = nc.declare_dram_parameter("tri", [128, 256], BF16, isOutput=False)
    yT = nc.declare_dram_parameter("yT", [D, T], F32, isOutput=True)

    with tile.TileContext(nc) as tc:
        with (
            tc.tile_pool(name="persist", bufs=1) as persist,
            tc.tile_pool(name="wpool", bufs=16) as wpool,
            tc.tile_pool(name="qtp", bufs=6) as qtp,
            tc.tile_pool(name="xtp", bufs=16) as xtp,
            tc.tile_pool(name="etp", bufs=8) as etp,
            tc.tile_pool(name="recp", bufs=2) as recp,
            tc.tile_pool(name="scrp", bufs=4) as scrp,
            tc.tile_pool(name="ytp", bufs=6) as ytp,
            tc.tile_pool(name="bigps", bufs=2, space="PSUM") as bigps,
            tc.tile_pool(name="smps", bufs=4, space="PSUM") as smps,
        ):
            # -------- persistent tiles --------
            kT = [persist.tile([128, T], BF16, tag=f"kt{i}", name=f"kt{i}")
                  for i in range(4)]
            v_sb = [persist.tile([128, NH * VW], BF16, tag=f"v{i}",
                                 name=f"v{i}") for i in range(KT)]
            osb = [persist.tile([128, T], BF16, tag=f"o{i}", name=f"o{i}")
                   for i in range(4)]
            wv_sb = [persist.tile([128, DL], BF16, tag=f"wv{k}",
                                  name=f"wv{k}") for k in range(8)]
            wp_sb = [persist.tile([128, D], BF16, tag=f"wp{k}",
                                  name=f"wp{k}") for k in range(4)]
            ones64 = persist.tile([1, 64], F32, tag="ones64", name="ones64")
            bq_sb = persist.tile([128, 4], F32, tag="bq", name="bq_sb")
            bk_sb = persist.tile([128, 4], F32, tag="bk", name="bk_sb")
            bv_bc = persist.tile([128, DL], F32, tag="bv", name="bv_bc")
            tri_sb = persist.tile([128, 256], BF16, tag="tri", name="tri_sb")

            def load_small_consts():
                nc.sync.dma_start(
                    out=bq_sb, in_=bq[:].rearrange("(a p) -> p a", p=128)
                )
                nc.sync.dma_start(
                    out=bk_sb, in_=bk[:].rearrange("(a p) -> p a", p=128)
                )
                nc.sync.dma_start(
                    out=bv_bc,
                    in_=bass.AP(tensor=bv[:].tensor, offset=0,
                                ap=[[0, 128], [1, DL]]),
                )
                nc.sync.dma_start(out=tri_sb, in_=tri[:, :])
                nc.vector.memset(ones64, 1.0)
                # ones columns of V (written once; V drains leave them)
                for tt in range(KT):
                    v3 = v_sb[tt].rearrange("p (h c) -> p h c", c=VW)
                    nc.vector.memset(v3[:, :, HD:HD + 1], 1.0)

            def load_big_consts():
                # wv streams on the SP queue behind xT; wp on the ACT queue
                # behind wk/wq (proj weights are needed much later)
                for k in range(8):
                    nc.sync.dma_start(
                        out=wv_sb[k], in_=wv[128 * k: 128 * k + 128, :]
                    )
                # wp on SP too: the ACT queue must drain before the first
                # exp can issue, and proj doesn't need wp until much later
                for k in range(4):
                    nc.sync.dma_start(
                        out=wp_sb[k], in_=wp[128 * k: 128 * k + 128, :]
                    )

            qts = [[None] * 4 for _ in range(CH)]

            # ---------------- QKV chain emitters ----------------
            def xt_unit(c):
                cs = slice(512 * c, 512 * c + 512)
                xts = []
                for k in range(8):
                    t_ = xtp.tile([128, 512], BF16, tag="xt", name="xt")
                    nc.sync.dma_start(
                        out=t_, in_=xT[128 * k: 128 * k + 128, cs]
                    )
                    xts.append(t_)
                return xts

            def w_unit(w_in, eng=None):
                eng = eng or nc.sync
                wts = []
                for k in range(8):
                    wt = wpool.tile([128, DL], BF16, tag="w", name="wt")
                    eng.dma_start(
                        out=wt, in_=w_in[128 * k: 128 * k + 128, :]
                    )
                    wts.append(wt)
                return wts

            def qk_chain(c, n, xts, wts, b_sb, is_q):
                acc = smps.tile([128, 512], F32, tag="sm", name="acc")
                for k in range(8):
                    nc.tensor.matmul(
                        acc,
                        wts[k][:, 128 * n: 128 * n + 128],
                        xts[k],
                        start=(k == 0), stop=(k == 7),
                    )
                qk_store(c, n, acc, b_sb, is_q)

            def qk_store(c, n, acc, b_sb, is_q):
                if is_q:
                    qt_n = qtp.tile([128, 512], BF16, tag="qt", name="qt")
                    nc.vector.tensor_scalar_add(
                        out=qt_n, in0=acc, scalar1=b_sb[:, n: n + 1]
                    )
                    qts[c][n] = qt_n
                else:
                    cs = slice(512 * c, 512 * c + 512)
                    nc.vector.tensor_scalar_add(
                        out=kT[n][:, cs], in0=acc, scalar1=b_sb[:, n: n + 1]
                    )

            def qk_chains_kmajor(c, xts, wts, b_sb, is_q):
                # all 4 chains advance together: at the k-th DMA landing,
                # 4 matmuls run, so the chunk-0 chains track the weight
                # stream instead of serializing chain-by-chain
                accs = [smps.tile([128, 512], F32, tag="sm", name="acc")
                        for _ in range(4)]
                for k in range(8):
                    for n in range(4):
                        nc.tensor.matmul(
                            accs[n],
                            wts[k][:, 128 * n: 128 * n + 128],
                            xts[k],
                            start=(k == 0), stop=(k == 7),
                        )
                for n in range(4):
                    qk_store(c, n, accs[n], b_sb, is_q)

            def v_chain(c, t4, xts):
                tt = 4 * c + t4
                acc = smps.tile([128, 512], F32, tag="sm", name="acc")
                for k in range(8):
                    nc.tensor.matmul(
                        acc,
                        xts[k][:, 128 * t4: 128 * t4 + 128],
                        wv_sb[k],
                        start=(k == 0), stop=(k == 7),
                    )
                v3 = v_sb[tt].rearrange("p (h c) -> p h c", c=VW)
                a3 = acc.rearrange("p (h c) -> p h c", c=HD)
                b3 = bv_bc.rearrange("p (h c) -> p h c", c=HD)
                nc.vector.tensor_add(v3[:, :, 0:HD], a3, b3)

            def qkv_units(c):
                """Emission units for chunk c's QKV; each is a callable."""
                state = {}

                def u_xw():
                    # interleave x and wq blocks on the SP queue so the
                    # k-th Q matmul's operands land together and the chain
                    # pipelines with the DMA stream instead of stalling on
                    # wq queued behind all 8 x blocks
                    cs = slice(512 * c, 512 * c + 512)
                    xts, wts = [], []
                    for k in range(8):
                        t_ = xtp.tile([128, 512], BF16, tag="xt", name="xt")
                        nc.sync.dma_start(
                            out=t_, in_=xT[128 * k: 128 * k + 128, cs])
                        xts.append(t_)
                        wt = wpool.tile([128, DL], BF16, tag="w", name="wt")
                        nc.sync.dma_start(
                            out=wt, in_=wq[128 * k: 128 * k + 128, :])
                        wts.append(wt)
                    state["xts"] = xts
                    state["wq"] = wts

                def u_wk():
                    state["wk"] = w_unit(wk)

                # units tagged (fn, has_pe_work): forced fills count only
                # PE-compute units (a DMA-trigger unit doesn't cover a PE
                # stall).  The DMA units lead the list so a single forced
                # fill at the chunk transition gets every stream going
                # ~15us earlier than pacing would.
                units = [(u_xw, False), (u_wk, False)]
                for n in range(4):
                    units.append((
                        lambda n=n: qk_chain(c, n, state["xts"],
                                             state["wq"], bq_sb, True),
                        True))
                for n in range(4):
                    units.append((
                        lambda n=n: qk_chain(c, n, state["xts"],
                                             state["wk"], bk_sb, False),
                        True))
                for t4 in range(4):
                    units.append((
                        lambda t4=t4: v_chain(c, t4, state["xts"]),
                        True))
                return units

            taccs = {}

            def tacc_partial(n):
                # first 3 contraction blocks of a tail proj chain; emitted
                # as late attn3 fillers (osb[0..2] chunk-3 columns are final
                # after groups 0..2), finished in the tail once osb[3] lands
                acc = smps.tile([128, 512], F32, tag="sm", name="acc")
                for k in range(3):
                    nc.tensor.matmul(
                        acc,
                        wp_sb[k][:, 128 * n: 128 * n + 128],
                        osb[k][:, 1536:2048],
                        start=(k == 0), stop=False,
                    )
                taccs[n] = acc

            def proj_chain(n, cp):
                acc = smps.tile([128, 512], F32, tag="sm", name="acc")
                for k in range(4):
                    nc.tensor.matmul(
                        acc,
                        wp_sb[k][:, 128 * n: 128 * n + 128],
                        osb[k][:, 512 * cp: 512 * cp + 512],
                        start=(k == 0), stop=(k == 3),
                    )
                yt = ytp.tile([128, 512], F32, tag="yt", name="yt")
                nc.vector.tensor_copy(yt, acc)
                nc.sync.dma_start(
                    out=yT[128 * n: 128 * n + 128,
                           512 * cp: 512 * cp + 512],
                    in_=yt,
                )

            # ---------------- attention ----------------
            def attn_group(g2, c, fill):
                qt, kt = qts[c][g2], kT[g2]
                ha, hb = 2 * g2, 2 * g2 + 1
                nb = 4 * (c + 1)
                av_a = smps.tile([VW, 512], F32, tag="sm", name="av_a")
                av_b = smps.tile([VW, 512], F32, tag="sm", name="av_b")
                ets = []

                def boff(b):
                    return 128 * (b - 4 * c) if b // 4 == c else 0

                strips = {}

                def s_a(b):
                    off = boff(b)
                    bs = slice(128 * b, 128 * b + 128)
                    strip = bigps.tile([128, 1024], F32, tag="strip",
                                       name="strip")
                    strips[b] = strip
                    nc.tensor.matmul(
                        strip[:, off:512],
                        kt[0:64, bs],
                        qt[0:64, off:512],
                        start=True, stop=True,
                    )

                def s_b(b):
                    off = boff(b)
                    bs = slice(128 * b, 128 * b + 128)
                    nc.tensor.matmul(
                        strips[b][:, 512 + off:1024],
                        kt[64:128, bs],
                        qt[64:128, off:512],
                        start=True, stop=True,
                    )

                def exp_mask(b):
                    off = boff(b)
                    strip = strips.pop(b)
                    et = etp.tile([128, 1024], BF16, tag="et", name="et")
                    if off == 0:
                        nc.scalar.activation(
                            et[:, 0:1024], strip[:, 0:1024], AF.Exp
                        )
                    else:
                        w_ = 512 - off
                        src_ap = bass.AP(
                            tensor=strip.tensor,
                            offset=strip.offset + off,
                            ap=[list(strip.ap[0]), [512, 2], [1, w_]],
                        )
                        dst_ap = bass.AP(
                            tensor=et.tensor,
                            offset=et.offset + off,
                            ap=[list(et.ap[0]), [512, 2], [1, w_]],
                        )
                        nc.scalar.activation(dst_ap, src_ap, AF.Exp)
                    if b // 4 == c:
                        m_dst = bass.AP(
                            tensor=et.tensor,
                            offset=et.offset + off,
                            ap=[list(et.ap[0]), [512, 2], [1, 128]],
                        )
                        m_tri = bass.AP(
                            tensor=tri_sb.tensor,
                            offset=tri_sb.offset,
                            ap=[list(tri_sb.ap[0]), [128, 2], [1, 128]],
                        )
                        nc.vector.tensor_tensor(
                            out=m_dst, in0=m_dst, in1=m_tri,
                            op=mybir.AluOpType.mult,
                        )
                    ets.append(et)

                def av(b):
                    et, off = ets[b], boff(b)
                    nc.tensor.matmul(
                        av_a[:, off:512],
                        v_sb[b][:, VW * ha: VW * ha + VW],
                        et[:, off:512],
                        start=(b == 0), stop=(b == nb - 1),
                    )
                    nc.tensor.matmul(
                        av_b[:, off:512],
                        v_sb[b][:, VW * hb: VW * hb + VW],
                        et[:, 512 + off:1024],
                        start=(b == 0), stop=(b == nb - 1),
                    )

                # blocks processed in pairs: each pair's 4 S matmuls are
                # emitted back-to-back (K=64 row-tiled pairs run packed on
                # disjoint row groups); AV trails by 2 pairs so its exp/mask
                # inputs are always long done when the PE reaches it
                # NOTE: the two quadrant matmuls of a block must stay
                # back-to-back - emitting them skewed across blocks
                # (A(b), B(b-1), ...) broke the HW's row-group
                # co-execution and cost ~110ns/block (measured)
                np_ = nb // 2
                for p in range(np_):
                    b0, b1 = 2 * p, 2 * p + 1
                    s_a(b0)
                    s_b(b0)
                    s_a(b1)
                    s_b(b1)
                    exp_mask(b0)
                    exp_mask(b1)
                    if p >= 1:
                        av(2 * p - 2)
                        av(2 * p - 1)
                        fill()
                    else:
                        # force >=2 filler units at the group start so the
                        # PE has matmul work while exp(b0) drains the strip
                        # buffer (only 2 bigps bufs -> s_mms(b2) must wait).
                        # At g2=0 count any units (the DMA-trigger units
                        # must go out anyway) but pump enough to reach the
                        # leading proj unit + the chunk's DMA issues.
                        # g2=0 (chunk transition): proj-lead + the 2 DMA
                        # units for attn1/2; attn0/attn3 have no stall-free
                        # PE lead resp. no DMA units, so just 2.  More
                        # would overflow the 4 smps accumulators (attn3) or
                        # block on just-issued DMAs (attn0).
                        fill(force=(3 if g2 == 0 and 0 < c < CH - 1
                                    else 2),
                             force_any=(g2 == 0))
                av(nb - 2)
                av(nb - 1)
                fill()

                # softmax finalize, decoupled: one DVE copy per head drains
                # the av bank (64 value rows + ones-row denominator) to SBUF
                # scratch so the PSUM bank frees in ~0.8us; the recip ->
                # broadcast -> scale chain then runs out of SBUF off the PE
                # critical path, with no fillers injected mid-chain (a yt
                # copy between the drain and the recip would stretch the
                # last group's osb latency).  Single-pass approx recip
                # (~5e-4 rel) is plenty: den >= 1 and the tolerance budget
                # is 2e-2.
                qs = slice(512 * c, 512 * c + 512)
                scr_a = scrp.tile([HD, 512], F32, tag="scr", name="scr_a")
                scr_b = scrp.tile([HD, 512], F32, tag="scr", name="scr_b")
                den_a = recp.tile([1, 512], F32, tag="den", name="den_a")
                den_b = recp.tile([1, 512], F32, tag="den", name="den_b")
                rec_a = recp.tile([1, 512], F32, tag="rec", name="rec_a")
                rec_b = recp.tile([1, 512], F32, tag="rec", name="rec_b")
                bc_a = recp.tile([64, 512], F32, tag="bc", name="bc_a")
                bc_b = recp.tile([64, 512], F32, tag="bc", name="bc_b")
                # den (ones-row) bounced to a base-partition-0 tile: the
                # custom-DVE recip reads garbage from base partition 64 on
                # hardware (CoreSim doesn't model the quirk)
                if c == CH - 1 and g2 == 3:
                    # last group: every ns of this chain is exposed before
                    # the tail's osb[3] matmuls.  Flush the reserved fillers
                    # (PE work for the window), drain the denominators first
                    # on DVE while the Scalar engine (idle - exps are done)
                    # drains the value rows in parallel, and keep the DVE
                    # queue free of the value drains so recip/mul run early.
                    fill(force=10 ** 6)
                    nc.vector.tensor_copy(den_a, av_a[HD:HD + 1, :])
                    nc.vector.reciprocal_approx_fast(out=rec_a, in_=den_a)
                    nc.gpsimd.partition_broadcast(bc_a, rec_a)
                    nc.vector.tensor_copy(den_b, av_b[HD:HD + 1, :])
                    nc.vector.reciprocal_approx_fast(out=rec_b, in_=den_b)
                    nc.gpsimd.partition_broadcast(bc_b, rec_b)
                    nc.vector.tensor_copy(scr_a, av_a[0:HD, :])
                    nc.vector.tensor_mul(osb[g2][0:64, qs], scr_a, bc_a)
                    nc.vector.tensor_copy(scr_b, av_b[0:HD, :])
                    nc.vector.tensor_mul(osb[g2][64:128, qs], scr_b, bc_b)
                else:
                    # drains first: the av PSUM banks free ~1.2us earlier,
                    # unblocking the next group's av accumulators; the
                    # recip/broadcast/mul chain into osb is off the
                    # critical path for non-last groups
                    nc.vector.tensor_copy(den_a, av_a[HD:HD + 1, :])
                    nc.vector.tensor_copy(scr_a, av_a[0:HD, :])
                    nc.vector.tensor_copy(den_b, av_b[HD:HD + 1, :])
                    nc.vector.tensor_copy(scr_b, av_b[0:HD, :])
                    nc.vector.reciprocal_approx_fast(out=rec_a, in_=den_a)
                    nc.gpsimd.partition_broadcast(bc_a, rec_a)
                    nc.vector.reciprocal_approx_fast(out=rec_b, in_=den_b)
                    nc.gpsimd.partition_broadcast(bc_b, rec_b)
                    nc.vector.tensor_mul(
                        osb[g2][0:64, qs], scr_a, bc_a
                    )
                    nc.vector.tensor_mul(
                        osb[g2][64:128, qs], scr_b, bc_b
                    )
                fill()

            # ---------------- chunk-0 QKV (startup-critical) ----------------
            # K first: its DVE bias-adds (which gate both kT for the S
            # matmuls and the smps bufs for the V chains) drain while the
            # Q matmuls run.  wk+xT stream in parallel on the two HWDGE
            # queues; wq behind wk on ACT; wv behind xT on SP.
            xts0 = xt_unit(0)
            load_small_consts()
            wks0 = w_unit(wk, eng=nc.scalar)
            wqs0 = w_unit(wq, eng=nc.scalar)
            load_big_consts()
            qk_chains_kmajor(0, xts0, wks0, bk_sb, False)
            qk_chains_kmajor(0, xts0, wqs0, bq_sb, True)
            for t4 in range(4):
                v_chain(0, t4, xts0)

            # ---------------- main pipeline ----------------
            for c in range(CH):
                with nc.named_scope(f"attn{c}"):
                    # proj(_, cp) only needs attention chunk cp complete, so
                    # it fills attn(cp+1) instead of crowding attn3.  One
                    # proj unit leads the list: it is the only stall-free PE
                    # unit at the chunk transition (osb + persistent wp), so
                    # the forced fill at g2=0/p=0 can cover the strip wait.
                    fillers = []
                    if c >= 1:
                        fillers += [((lambda n=0, cp=c - 1:
                                      proj_chain(n, cp)), True)]
                    if c < CH - 1:
                        fillers += qkv_units(c + 1)
                    if c >= 1:
                        fillers += [
                            ((lambda n=n, cp=c - 1: proj_chain(n, cp)), True)
                            for n in range(1, 8)
                        ]
                    if c == CH - 1:
                        fillers += [
                            ((lambda n=n: tacc_partial(n)), True)
                            for n in range(2)
                        ]
                    # attn3 over-provisions fill points so ~2 units remain
                    # for the last group's finalize flush (PE starves there
                    # otherwise - exp stream is done, tail waits on osb[3])
                    total_pts = 4 * (2 * (c + 1) + 2)
                    if c == CH - 1:
                        total_pts += 4
                    state = {"pts": 0, "emitted": 0}

                    def fill(state=state, fillers=fillers,
                             total_pts=total_pts, force=0, force_any=False):
                        state["pts"] += 1
                        goal = state["pts"] / total_pts
                        n_ = len(fillers)
                        while (state["emitted"] < n_
                               and (state["emitted"] / n_ <= goal
                                    or force > 0)):
                            fn, is_pe = fillers[state["emitted"]]
                            fn()
                            state["emitted"] += 1
                            if is_pe or force_any:
                                force -= 1

                    for g2 in range(4):
                        attn_group(g2, c, fill)
                    while state["emitted"] < len(fillers):
                        fillers[state["emitted"]][0]()
                        state["emitted"] += 1

            with nc.named_scope("tail"):
                # all 8 chains pre-compute their k<3 partials before any
                # k=3 matmul: n=2,3 reuse the freed av smps banks, n=4..7
                # borrow the bigps strip banks (attention is done - each
                # [128,1024] strip hosts two accumulators), so only the 8
                # k=3 matmuls wait on osb[3]
                tacc_partial(2)
                tacc_partial(3)
                tstrips = [bigps.tile([128, 1024], F32, tag="strip",
                                      name="tstrip") for _ in range(2)]
                for n in range(4, 8):
                    acc = tstrips[(n - 4) // 2][:, 512 * (n % 2):
                                                512 * (n % 2) + 512]
                    for k in range(3):
                        nc.tensor.matmul(
                            acc,
                            wp_sb[k][:, 128 * n: 128 * n + 128],
                            osb[k][:, 1536:2048],
                            start=(k == 0), stop=False,
                        )
                    taccs[n] = acc
                for n in range(8):
                    nc.tensor.matmul(
                        taccs[n],
                        wp_sb[3][:, 128 * n: 128 * n + 128],
                        osb[3][:, 1536:2048],
                        start=False, stop=True,
                    )
                    yt = ytp.tile([128, 512], F32, tag="yt", name="yt")
                    nc.vector.tensor_copy(yt, taccs[n])
                    nc.sync.dma_start(
                        out=yT[128 * n: 128 * n + 128, 1536:2048],
                        in_=yt,
                    )
    return nc


_prog = None


def _get_program():
    global _prog
    if _prog is None:
        _prog = build(bacc.Bacc(None))
        _prog.finalize()
    return _prog


def make_in_maps(x, w_qkv, b_qkv, w_proj):
    x = np.ascontiguousarray(np.asarray(x, np.float32))
    w_qkv = np.asarray(w_qkv, np.float32)
    b_qkv = np.asarray(b_qkv, np.float32)
    w_proj = np.asarray(w_proj, np.float32)
    tri1 = np.triu(np.ones((128, 128), np.float32))
    tri2 = np.concatenate([tri1, tri1], 1).astype(ml_dtypes.bfloat16)
    in_maps = []
    for core in range(8):
        b, g = divmod(core, 2)
        gs = slice(DL * g, DL * g + DL)
        gk = slice(D + DL * g, D + DL * g + DL)
        gv = slice(2 * D + DL * g, 2 * D + DL * g + DL)
        in_maps.append({
            "xT": np.ascontiguousarray(x[b].T).astype(ml_dtypes.bfloat16),
            "wq": (np.ascontiguousarray(w_qkv[:, gs])
                   * np.float32(0.125)).astype(ml_dtypes.bfloat16),
            "wk": np.ascontiguousarray(
                w_qkv[:, gk]).astype(ml_dtypes.bfloat16),
            "wv": np.ascontiguousarray(
                w_qkv[:, gv]).astype(ml_dtypes.bfloat16),
            "bq": np.ascontiguousarray(b_qkv[gs]) * np.float32(0.125),
            "bk": np.ascontiguousarray(b_qkv[gk]),
            "bv": np.ascontiguousarray(b_qkv[gv]),
            "wp": np.ascontiguousarray(
                w_proj[DL * g: DL * g + DL, :]).astype(ml_dtypes.bfloat16),
            "tri": tri2,
        })
    return in_maps


def combine_outputs(results, b_proj):
    b_proj = np.asarray(b_proj, np.float32)
    y = np.empty((B, T, D), np.float32)
    for b in range(B):
        yt = results[2 * b]["yT"] + results[2 * b + 1]["yT"]
        y[b] = yt.T + b_proj
    return y


def kernel(x, w_qkv, b_qkv, w_proj, b_proj, **run_kwargs):
    in_maps = make_in_maps(x, w_qkv, b_qkv, w_proj)
    r = run_bass_kernel_spmd(_get_program(), in_maps,
                             core_ids=list(range(8)), **run_kwargs)
    out = combine_outputs(r.results, b_proj)
    kernel.last_result = r
    return out
